# revision 25
# baseline (speedup 1.0000x reference)
"""Trainium2 Bass kernel for nn_FBResEEGMoE (FIR filterbank + depthwise convs +
BN/ReLU + sconv + MoE gate/experts + FC + log_softmax).

Strategy (pure data parallelism, 8 cores x 32 samples):
  * The whole chain x -> FIR -> {conv3,conv5,conv7} -> concat/stride3-mean -> BN
    is linear per (band, channel): build the exact banded map L[nb] (1000x1000)
    on the host from the weights.
  * relu(z) = (z + |z|)/2, so pooled-sconv output
        hpool[nb,p,s] = sum_c (w[nb,c]/250) * sum_{t' in pool} (z + |z|)
    - sum z   : composed into tiny "PL" matmuls (linear path)
    - sum |z| : ONE pass over the big intermediate, done by DVE
                tensor_reduce(abs) directly from PSUM (fused ReLU+pool)
  * Big matmuls in fp16 (1 cyc/col on PE, fp32 PSUM accumulate); everything
    downstream of the pooling (gate softmax/top-3, experts, mixing, FC,
    log_softmax) is tiny and stays fp32 on-chip.

Outputs per core: out1 = mixed expert output [48(o), 8(p), 32(s)] and
out2 = logp [32(s), 9]; host reassembles (feats, logp) for the full batch.
"""

import sys

for _p in ("/opt/trn_rl_repo",):
    if _p not in sys.path:
        sys.path.insert(0, _p)

import numpy as np

import concourse.bass as bass
import concourse.bacc as bacc
import concourse.tile as tile
import concourse.mybir as mybir
from concourse.bass_utils import run_bass_kernel_spmd

F16 = mybir.dt.float16
F32 = mybir.dt.float32
AX = mybir.AxisListType
ALU = mybir.AluOpType
ACTF = mybir.ActivationFunctionType

# problem constants
B, C, T = 256, 22, 1000
NB, E, K = 6, 6, 3
POOL, NPOOL = 125, 8
MNB, NCLS = 48, 9
NUM_TAPS = 21
EPS = 1e-5

NCORES, S = 8, 32
CS = C * S                       # 704 flattened (c,s), col = c*S + s
CHUNKS = [(i * 128, min(128, CS - i * 128)) for i in range((CS + 127) // 128)]  # 6 chunks
NCH = len(CHUNKS)


# ----------------------------------------------------------------------------
# geometry (value-independent: derived from conv structure only)
# ----------------------------------------------------------------------------
def _support(tp):
    """Exact tau-support [lo, hi] of output t' (union over bands)."""
    taus = set()
    for i in (3 * tp, 3 * tp + 1, 3 * tp + 2):
        r, pos = i // T, i % T
        k = (3, 5, 7)[r]
        for t in range(max(0, pos - k // 2), min(T - 1, pos + k // 2) + 1):
            taus.add(max(0, t - 10))
            taus.add(min(T - 1, t + 10))
    return min(taus), max(taus)


RL = 28                     # t'-slots per (window, region) run
TLW = 3 * RL                # 84 column-slots per band per window
WROWS = 112                 # tau rows per window (+1 ones row)
ACT_FRAC = 0.55             # fraction of windows evacuated via ACT Abs lane


class Geometry:
    """Adaptive window placement + run/slot assignment for the abs-reduce."""

    def __init__(self):
        lohi = [_support(tp) for tp in range(T)]
        self.specials = [tp for tp in range(T) if lohi[tp][1] - lohi[tp][0] + 1 > WROWS]
        assert self.specials == [333, 666], self.specials
        sset = set(self.specials)
        region = {}
        reg_t = [[], [], []]
        for tp in range(T):
            if tp in sset:
                continue
            r = (3 * tp) // T
            region[tp] = r
            reg_t[r].append(tp)
        # adaptive window placement
        self.tau0 = []
        self.runs = []            # runs[w][r] = list of t'
        ptr = [0, 0, 0]
        while any(ptr[r] < len(reg_t[r]) for r in range(3)):
            cand = [lohi[reg_t[r][ptr[r]]][0] for r in range(3) if ptr[r] < len(reg_t[r])]
            t0 = min(cand)
            wrun = []
            for r in range(3):
                run = []
                while (ptr[r] < len(reg_t[r]) and len(run) < RL):
                    tp = reg_t[r][ptr[r]]
                    lo, hi = lohi[tp]
                    if lo < t0 or hi > t0 + WROWS - 1:
                        break
                    if run and tp // POOL != run[0] // POOL:
                        break
                    run.append(tp)
                    ptr[r] += 1
                wrun.append(run)
            assert any(wrun), f"no progress at tau0={t0}"
            self.tau0.append(t0)
            self.runs.append(wrun)
        self.NW = len(self.tau0)
        assert all(region[tp] is not None for tp in region)
        n_assigned = sum(len(run) for wr in self.runs for run in wr)
        assert n_assigned == T - len(self.specials), n_assigned
        # slot assignment: for each window solve arithmetic-progression slots
        # for the 3 run-sums; tails (pool-crossing) get free slots.
        # Iterate SL until a consistent assignment exists.
        for SL in range(4, 40):
            ok = self._assign(SL)
            if ok:
                break
        assert ok, "no slot assignment found"
        self.SL = SL

    def _assign(self, SL):
        taken = [set() for _ in range(NPOOL)]
        self.call1 = []              # per w: (s0, D)
        self.tails = []              # per w: list of (r, ci, poolB, slotB, poolA, slotC)

        def alloc(pool, slot=None):
            if slot is None:
                cands = [s for s in range(SL) if s not in taken[pool]]
                if not cands:
                    return None
                slot = cands[0]
            elif slot in taken[pool]:
                return None
            taken[pool].add(slot)
            return slot

        for w in range(self.NW):
            wr = self.runs[w]
            pools = [wr[r][0] // POOL if wr[r] else None for r in range(3)]
            sol = None
            for slot0 in range(SL):
                p0 = pools[0] if pools[0] is not None else 0
                if slot0 in taken[p0]:
                    continue
                s0 = p0 * SL + slot0
                for D in range(1, 3 * SL + 1):
                    cols = [s0 + r * D for r in range(3)]
                    if cols[2] >= NPOOL * SL:
                        break
                    good = True
                    for r in range(3):
                        pr, sr = divmod(cols[r], SL)
                        if pools[r] is not None and pr != pools[r]:
                            good = False
                            break
                        if sr in taken[pr]:
                            good = False
                            break
                    if good:
                        sol = (s0, D, cols)
                        break
                if sol:
                    break
            if sol is None:
                return False
            s0, D, cols = sol
            for c in cols:
                taken[c // SL].add(c % SL)
            self.call1.append((s0, D))
            wt = []
            for r in range(3):
                run = self.runs[w][r]
                if not run:
                    continue
                pA = run[0] // POOL
                ci = None
                for j, tp in enumerate(run):
                    if tp // POOL != pA:
                        ci = j
                        break
                if ci is not None:
                    pB = run[ci] // POOL
                    sB = alloc(pB)
                    sC = alloc(pA)
                    if sB is None or sC is None:
                        return False
                    wt.append((r, ci, pB, sB, pA, sC))
            self.tails.append(wt)
        self.sp_slot = {}
        for pool_id in (2, 5):
            s = alloc(pool_id)
            if s is None:
                return False
            self.sp_slot[pool_id] = s
        return True


GEO = Geometry()
SL = GEO.SL
NW = GEO.NW


# ----------------------------------------------------------------------------
# host-side constant construction (depends on weight values)
# ----------------------------------------------------------------------------
def _build_L(fir, w3, w5, w7, scale1):
    """L2[nb] (T x T): BN-scaled pre-ReLU map, h_pre[nb,c,t'] = L2[nb] @ x[c]."""
    idx = np.arange(T)
    Ls = np.zeros((NB, T, T))
    for nb in range(NB):
        F = np.zeros((T, T))
        for k in range(NUM_TAPS):
            tau = idx + k - 10
            v = (tau >= 0) & (tau < T)
            F[idx[v], tau[v]] += float(fir[nb, k])

        def convF(wv):
            kk = len(wv)
            Ck = np.zeros_like(F)
            for j in range(kk):
                src = idx + j - kk // 2
                v = (src >= 0) & (src < T)
                Ck[idx[v]] += float(wv[j]) * F[src[v]]
            return Ck

        CONCAT = np.concatenate([convF(w3[nb]), convF(w5[nb]), convF(w7[nb])], axis=0)
        Ls[nb] = (CONCAT[3 * idx] + CONCAT[3 * idx + 1] + CONCAT[3 * idx + 2]) / 3.0
    return Ls * scale1[:, None, None]


def build_consts(inputs):
    """All device input arrays (shared across cores) from the raw weights."""
    fir = np.asarray(inputs["fir_kernels"], np.float64)
    w3 = np.asarray(inputs["pconv_w3"], np.float64)[:, 0, 0, :]
    w5 = np.asarray(inputs["pconv_w5"], np.float64)[:, 0, 0, :]
    w7 = np.asarray(inputs["pconv_w7"], np.float64)[:, 0, 0, :]
    scale1 = np.asarray(inputs["bn1_g"], np.float64) / np.sqrt(np.asarray(inputs["bn1_v"], np.float64) + EPS)
    beta1 = np.asarray(inputs["bn1_b"], np.float64) - np.asarray(inputs["bn1_m"], np.float64) * scale1
    sconv = np.asarray(inputs["sconv_w"], np.float64)[:, 0, :, 0]     # (NB, C)
    gate_w = np.asarray(inputs["gate_w"], np.float64)
    gate_b = np.asarray(inputs["gate_b"], np.float64)
    exp_w = np.asarray(inputs["exp_w"], np.float64)
    exp_b = np.asarray(inputs["exp_b"], np.float64)
    escale = np.asarray(inputs["ebn_g"], np.float64) / np.sqrt(np.asarray(inputs["ebn_v"], np.float64) + EPS)
    ebeta = np.asarray(inputs["ebn_b"], np.float64) - np.asarray(inputs["ebn_m"], np.float64) * escale
    fc_w = np.asarray(inputs["fc_w"], np.float64)
    fc_b = np.asarray(inputs["fc_b"], np.float64)

    L2 = _build_L(fir, w3, w5, w7, scale1)
    bh = beta1                                            # bias of z per band

    # R blocks: [128(row), NW, 6(nb), 3(run), RL]
    R = np.zeros((128, NW, NB, 3, RL), np.float32)
    for w in range(NW):
        t0 = GEO.tau0[w]
        hi = min(T, t0 + WROWS)
        for nb in range(NB):
            for r in range(3):
                run = GEO.runs[w][r]
                if not run:
                    continue
                R[0, w, nb, r, :len(run)] = bh[nb]
                R[1:1 + hi - t0, w, nb, r, :len(run)] = L2[nb, run, t0:hi].T
    # specials: [128, 2(part), 6(nb), 2(which)]
    RS = np.zeros((128, 2, NB, 2), np.float32)
    tlast = GEO.tau0[NW - 1]
    for si, tp in enumerate(GEO.specials):
        for nb in range(NB):
            RS[0, 0, nb, si] = bh[nb]
            RS[1:1 + WROWS, 0, nb, si] = L2[nb, tp, 0:WROWS]
            RS[1:1 + T - tlast, 1, nb, si] = L2[nb, tp, tlast:T]
            assert np.all(L2[nb, tp, WROWS:tlast] == 0.0)

    # PL blocks: [128, NW, 48] (pool-summed rows incl. bias; specials folded)
    PLw = np.zeros((128, NW, MNB), np.float64)
    for w in range(NW):
        t0 = GEO.tau0[w]
        hi = min(T, t0 + WROWS)
        for nb in range(NB):
            for r in range(3):
                for tp in GEO.runs[w][r]:
                    col = nb * NPOOL + tp // POOL
                    PLw[0, w, col] += bh[nb]
                    PLw[1:1 + hi - t0, w, col] += L2[nb, tp, t0:hi]
    for si, tp in enumerate(GEO.specials):
        for nb in range(NB):
            col = nb * NPOOL + tp // POOL
            PLw[0, 0, col] += bh[nb]
            PLw[1:1 + WROWS, 0, col] += L2[nb, tp, 0:WROWS]
            PLw[1:1 + T - tlast, NW - 1, col] += L2[nb, tp, tlast:T]

    # Wpat: [128, NCH, 2, 96] fp32 : delta(s,s') * sconv[3h+nbr, c]/250
    Wp = np.zeros((128, NCH, 2, 96), np.float32)
    for ch, (cs0, P) in enumerate(CHUNKS):
        for p in range(P):
            c, s = divmod(cs0 + p, S)
            for h in range(2):
                for nbr in range(3):
                    Wp[p, ch, h, nbr * S + s] = sconv[3 * h + nbr, c] / 250.0

    gateW = np.zeros((7, MNB), np.float32)
    gateW[:6] = (gate_w.T / NPOOL)
    gateW[6] = gate_b

    expW = np.zeros((7, 3, 96), np.float32)
    for ch3 in range(3):
        for j in range(96):
            eo = ch3 * 96 + j
            e, o = divmod(eo, MNB)
            expW[:6, ch3, j] = exp_w[e, o, :] * escale[e, o]
            expW[6, ch3, j] = exp_b[e, o] * escale[e, o] + ebeta[e, o]

    SEL = np.zeros((96, 3, MNB), np.float32)
    for ch3 in range(3):
        for j in range(96):
            SEL[j, ch3, (ch3 * 96 + j) % MNB] = 1.0

    fcW = np.zeros((49, NPOOL, NCLS), np.float32)
    for p in range(NPOOL):
        for o in range(MNB):
            fcW[o, p, :] = fc_w[:, o * NPOOL + p]
    fcW[48, 0, :] = fc_b

    iden = np.eye(32, dtype=np.float32)

    return {
        "rmat": R.reshape(128, -1).astype(np.float16),
        "rspec": RS.reshape(128, -1).astype(np.float16),
        "plmat": PLw.reshape(128, -1).astype(np.float16),
        "wpat": Wp.reshape(128, -1),
        "gatew": gateW,
        "expw": expW.reshape(7, -1),
        "selm": SEL.reshape(96, -1),
        "fcw": fcW.reshape(49, -1),
        "iden": iden,
        "onesrow": np.ones((1, NPOOL * S), np.float32),
    }


def build_xtw(xcore):
    """Per-core x windows: [128, NW, CS] fp16 (row 0 = ones, rows 1.. = x^T)."""
    xt = np.zeros((128, NW, CS), np.float16)
    xt[0] = 1.0
    xpad = np.zeros((S, C, GEO.tau0[-1] + WROWS), np.float32)
    xpad[:, :, :T] = xcore
    for w in range(NW):
        t0 = GEO.tau0[w]
        blk = xpad[:, :, t0:t0 + WROWS]                   # (S, C, 112)
        xt[1:1 + WROWS, w, :] = blk.transpose(2, 1, 0).reshape(WROWS, CS).astype(np.float16)
    return xt.reshape(128, -1)


# ----------------------------------------------------------------------------
# bass program
# ----------------------------------------------------------------------------

def build_program():
    nc = bacc.Bacc("TRN2", target_bir_lowering=False, debug=False)

    din = {}
    for name, shape, dt in [
        ("xtw", (128, NW * CS), F16),
        ("rmat", (128, NW * NB * 3 * RL), F16),
        ("rspec", (128, 2 * NB * 2), F16),
        ("plmat", (128, NW * MNB), F16),
        ("wpat", (128, NCH * 2 * 96), F32),
        ("gatew", (7, MNB), F32),
        ("expw", (7, 3 * 96), F32),
        ("selm", (96, 3 * MNB), F32),
        ("fcw", (49, NPOOL * NCLS), F32),
        ("iden", (32, 32), F32),
        ("onesrow", (1, NPOOL * S), F32),
    ]:
        din[name] = nc.dram_tensor(name, list(shape), dt, kind="ExternalInput").ap()
    g6d = nc.dram_tensor("g6scratch", [E, S], F32, kind="Internal").ap()
    out1 = nc.dram_tensor("out1", [MNB, NPOOL * S], F32, kind="ExternalOutput").ap()
    out2 = nc.dram_tensor("out2", [S, NCLS], F32, kind="ExternalOutput").ap()

    with tile.TileContext(nc) as tc:
        with (
            tc.tile_pool(name="consts", bufs=1) as cpool,
            tc.tile_pool(name="ps", bufs=3, space="PSUM") as ps_pool,
            tc.tile_pool(name="acc", bufs=1, space="PSUM") as acc_pool,
            tc.tile_pool(name="cc", bufs=1, space="PSUM") as cc_pool,
            tc.tile_pool(name="sb", bufs=2) as sbp,
            tc.tile_pool(name="small", bufs=1) as sm,
        ):
            # ---- load constants ----
            sb = {}
            for name, ap in din.items():
                t = cpool.tile(list(ap.shape), ap.dtype, tag=name)
                nc.sync.dma_start(t[:], ap)
                sb[name] = t
            xtw = sb["xtw"][:].rearrange("p (w n) -> p w n", w=NW)         # [128, NW, 704]
            rmat = sb["rmat"][:].rearrange("p (w n) -> p w n", w=NW)         # [128, NW, 504]
            plm = sb["plmat"][:].rearrange("p (w n) -> p w n", w=NW)
            rsp = sb["rspec"][:].rearrange("p (k n w2) -> p k n w2", k=2, n=NB)
            wpat = sb["wpat"][:].rearrange("p (c h m) -> p c h m", c=NCH, h=2)
            expw = sb["expw"][:].rearrange("p (c m) -> p c m", c=3)
            selm = sb["selm"][:].rearrange("p (c m) -> p c m", c=3)
            fcw = sb["fcw"][:].rearrange("p (k n) -> p k n", k=NPOOL)

            cc_ps = [cc_pool.tile([96, MNB], F32, tag=f"cc{h}", name=f"cc{h}") for h in range(2)]

            # ================= main loop: chunks x windows =================
            for ch, (cs0, P) in enumerate(CHUNKS):
                lin_ps = acc_pool.tile([P, MNB], F32, tag="lin")
                sp_ps = acc_pool.tile([P, NB * 2], F32, tag="sp")
                partials = sbp.tile([128, MNB * SL], F32, tag="partials")
                nc.vector.memset(partials[:], 0.0)
                pview = partials[:].rearrange("p (n sl) -> p n sl", sl=SL)

                for w in range(NW):
                    lhsT = xtw[:, w, cs0:cs0 + P]
                    nc.tensor.matmul(lin_ps[:], lhsT, plm[:, w, :],
                                     start=(w == 0), stop=(w == NW - 1))
                    if w == 0:
                        nc.tensor.matmul(sp_ps[:], lhsT,
                                         rsp[:, 0].rearrange("p n w2 -> p (n w2)"),
                                         start=True, stop=False)
                    if w == NW - 1:
                        nc.tensor.matmul(sp_ps[:], lhsT,
                                         rsp[:, 1].rearrange("p n w2 -> p (n w2)"),
                                         start=False, stop=True)
                    pst = ps_pool.tile([P, NB * TLW], F32, tag="ps", name="pst")
                    nc.tensor.matmul(pst[:], lhsT, rmat[:, w, :], start=True, stop=True)
                    pg = partials[:P].rearrange("p (n x) -> p n x", n=NB)
                    pv = pst[:].rearrange("p (n r j) -> p n r j", n=NB, r=3)
                    s0, D = GEO.call1[w]
                    nc.vector.tensor_reduce(
                        pg[:, :, s0:s0 + 2 * D + 1:D], pv,
                        axis=AX.X, op=ALU.add, apply_absolute_value=True)
                    # tails: pool-crossing corrections (always fp32 PSUM source)
                    for (r, ci, pB, sB, pA, sC) in GEO.tails[w]:
                        tin = pv[:, :, r, ci:RL]
                        nc.vector.tensor_reduce(
                            pg[:, :, pB * SL + sB:pB * SL + sB + 1], tin,
                            axis=AX.X, op=ALU.add, apply_absolute_value=True)
                        nc.vector.tensor_reduce(
                            pg[:, :, pA * SL + sC:pA * SL + sC + 1], tin,
                            axis=AX.X, op=ALU.add, apply_absolute_value=True,
                            negate=True)
                # specials reduce (which=0 -> pool2, which=1 -> pool5)
                for si, pool_id in ((0, 2), (1, 5)):
                    col = pool_id * SL + GEO.sp_slot[pool_id]
                    nc.vector.tensor_reduce(
                        partials[:P].rearrange("p (n x) -> p n x", n=NB)[:, :, col:col + 1],
                        sp_ps[:].rearrange("p (n w2) -> p n w2", w2=2)[:, :, si:si + 1],
                        axis=AX.X, op=ALU.add, apply_absolute_value=True)
                # combine: hpS = sum_slots partials + lin
                hpS = sbp.tile([P, MNB], F32, tag="hpS")
                nc.vector.tensor_reduce(hpS[:], pview[:P], axis=AX.X, op=ALU.add)
                nc.vector.tensor_tensor(hpS[:], hpS[:], lin_ps[:], ALU.add)
                for h in range(2):
                    nc.tensor.matmul(cc_ps[h][:], wpat[:P, ch, h, :], hpS[:],
                                     start=(ch == 0), stop=(ch == NCH - 1))

            # ========================= head =========================
            ccS = [sm.tile([96, MNB], F32, tag=f"ccS{h}", name=f"ccS{h}") for h in range(2)]
            for h in range(2):
                nc.vector.tensor_copy(ccS[h][:], cc_ps[h][:])
            hpT = sm.tile([7, NPOOL * S], F32, tag="hpT")   # rows 0-5 hpoolT, row6 ones
            nc.sync.dma_start(hpT[6:7, :], din["onesrow"])
            for h in range(2):
                for nbr in range(3):
                    nb = 3 * h + nbr
                    src = ccS[h][nbr * S:(nbr + 1) * S, nb * NPOOL:(nb + 1) * NPOOL]  # [32(s), 8(p)]
                    dst = hpT[nb:nb + 1, :].rearrange("one (s p) -> one s p", s=S)
                    nc.sync.dma_start(dst, src)
            # gate tile [7, 32]: rows 0-5 = sum_p hpoolT, row 6 ones
            gt = sm.tile([7, S], F32, tag="gt")
            nc.sync.dma_start(gt[6:7, :], din["onesrow"][:, 0:S])
            nc.vector.tensor_reduce(
                gt[0:6, :], hpT[0:6, :].rearrange("n (s p) -> n s p", s=S),
                axis=AX.X, op=ALU.add)
            glog = ps_pool.tile([S, MNB], F32, tag="ps")
            nc.tensor.matmul(glog[:], gt[:], sb["gatew"][:], start=True, stop=True)
            # softmax
            m1 = sm.tile([S, 1], F32, tag="m1")
            nc.vector.tensor_reduce(m1[:], glog[:], axis=AX.X, op=ALU.max)
            nm1 = sm.tile([S, 1], F32, tag="nm1")
            nc.vector.tensor_scalar_mul(nm1[:], m1[:], -1.0)
            ex = sm.tile([S, MNB], F32, tag="ex")
            nc.scalar.activation(ex[:], glog[:], ACTF.Exp, bias=nm1[:], scale=1.0)
            sm1 = sm.tile([S, 1], F32, tag="sm1")
            nc.vector.tensor_reduce(sm1[:], ex[:], axis=AX.X, op=ALU.add)
            rs1 = sm.tile([S, 1], F32, tag="rs1")
            nc.vector.reciprocal(rs1[:], sm1[:])
            gw = sm.tile([S, MNB], F32, tag="gw")
            nc.vector.tensor_scalar(gw[:], ex[:], rs1[:], None, op0=ALU.mult)
            # top-3 mask
            msum = sm.tile([S, MNB], F32, tag="msum")
            nc.vector.memset(msum[:], 0.0)
            cur = gw
            for it in range(K):
                mx = sm.tile([S, 1], F32, tag=f"mx{it}")
                nc.vector.tensor_reduce(mx[:], cur[:], axis=AX.X, op=ALU.max)
                mk = sm.tile([S, MNB], F32, tag=f"mk{it}")
                nc.vector.tensor_scalar(mk[:], cur[:], mx[:], None, op0=ALU.is_ge)
                nc.vector.tensor_tensor(msum[:], msum[:], mk[:], ALU.add)
                if it < K - 1:
                    nxt = sm.tile([S, MNB], F32, tag=f"cur{it}")
                    big = sm.tile([S, MNB], F32, tag=f"big{it}")
                    nc.vector.tensor_scalar_mul(big[:], mk[:], 1e30)
                    nc.vector.tensor_tensor(nxt[:], cur[:], big[:], ALU.subtract)
                    cur = nxt
            gwm = sm.tile([S, MNB], F32, tag="gwm")
            nc.vector.tensor_tensor(gwm[:], gw[:], msum[:], ALU.mult)
            den = sm.tile([S, 1], F32, tag="den")
            nc.vector.tensor_reduce(den[:], gwm[:], axis=AX.X, op=ALU.add)
            rden = sm.tile([S, 1], F32, tag="rden")
            nc.vector.reciprocal(rden[:], den[:])
            gwn = sm.tile([S, MNB], F32, tag="gwn")
            nc.vector.tensor_scalar(gwn[:], gwm[:], rden[:], None, op0=ALU.mult)
            # gw6T [6, 32] via PE transpose
            g6ps = ps_pool.tile([E, S], F32, tag="ps")
            nc.tensor.matmul(g6ps[:], gwn[:, 0:E], sb["iden"][:], is_transpose=True)
            gw6T = sm.tile([E, S], F32, tag="gw6T")
            nc.vector.tensor_copy(gw6T[:], g6ps[:])
            nc.sync.dma_start(g6d, gw6T[:])
            # experts + mixing
            mix_ps = ps_pool.tile([MNB, NPOOL * S], F32, tag="ps")
            for ch3 in range(3):
                yp_ps = ps_pool.tile([96, NPOOL * S], F32, tag="ps")
                nc.tensor.matmul(yp_ps[:], expw[:, ch3, :], hpT[:], start=True, stop=True)
                ypR = sbp.tile([96, NPOOL * S], F32, tag="ypR")
                nc.scalar.activation(ypR[:], yp_ps[:], ACTF.Relu)
                G = sbp.tile([96, NPOOL * S], F32, tag="G")
                for er in range(2):
                    e = 2 * ch3 + er
                    # G stored (p,s)-major: innermost s is contiguous on both sides
                    srcb = g6d[e:e + 1, :].unsqueeze(1).broadcast_to([48, NPOOL, S])
                    dstb = G[er * 48:(er + 1) * 48, :].rearrange("o (p s) -> o p s", p=NPOOL)
                    nc.sync.dma_start(dstb, srcb)
                Gm = sbp.tile([96, NPOOL * S], F32, tag="Gm")
                ypv = ypR[:].rearrange("o (s p) -> o s p", s=S)
                Gv = G[:].rearrange("o (p s) -> o p s", p=NPOOL).transpose([0, 2, 1])
                Gmv = Gm[:].rearrange("o (s p) -> o s p", s=S)
                nc.vector.tensor_tensor(Gmv, ypv, Gv, ALU.mult)
                nc.tensor.matmul(mix_ps[:], selm[:, ch3, :], Gm[:],
                                 start=(ch3 == 0), stop=(ch3 == 2))
            mixS = sm.tile([49, NPOOL * S], F32, tag="mixS")
            nc.sync.dma_start(mixS[48:49, :], din["onesrow"])
            nc.vector.tensor_copy(mixS[0:48, :], mix_ps[:])
            nc.sync.dma_start(out1, mixS[0:48, :])
            # fc
            lg_ps = ps_pool.tile([S, NCLS], F32, tag="ps")
            mixv = mixS[:].rearrange("k (s p) -> k s p", s=S)
            for p in range(NPOOL):
                nc.tensor.matmul(lg_ps[:], mixv[:, :, p], fcw[:, p, :],
                                 start=(p == 0), stop=(p == NPOOL - 1))
            m2 = sm.tile([S, 1], F32, tag="m2")
            nc.vector.tensor_reduce(m2[:], lg_ps[:], axis=AX.X, op=ALU.max)
            nm2 = sm.tile([S, 1], F32, tag="nm2")
            nc.vector.tensor_scalar_mul(nm2[:], m2[:], -1.0)
            ex2 = sm.tile([S, NCLS], F32, tag="ex2")
            nc.scalar.activation(ex2[:], lg_ps[:], ACTF.Exp, bias=nm2[:], scale=1.0)
            s2 = sm.tile([S, 1], F32, tag="s2")
            nc.vector.tensor_reduce(s2[:], ex2[:], axis=AX.X, op=ALU.add)
            ls2 = sm.tile([S, 1], F32, tag="ls2")
            nc.scalar.activation(ls2[:], s2[:], ACTF.Ln)
            lgp = sm.tile([S, NCLS], F32, tag="lgp")
            nc.vector.tensor_scalar(lgp[:], lg_ps[:], m2[:], ls2[:],
                                    op0=ALU.subtract, op1=ALU.subtract)
            nc.sync.dma_start(out2, lgp[:])

    nc.compile()
    return nc


_NC = None


def _get_nc():
    global _NC
    if _NC is None:
        _NC = build_program()
    return _NC


def kernel(**inputs):
    x = np.asarray(inputs["x"], np.float32)
    consts = build_consts(inputs)
    nc = _get_nc()
    in_maps = []
    for core in range(NCORES):
        m = dict(consts)
        m["xtw"] = build_xtw(x[core * S:(core + 1) * S])
        in_maps.append(m)
    res = run_bass_kernel_spmd(nc, in_maps, list(range(NCORES))).results
    feats = np.zeros((B, MNB * NPOOL), np.float32)
    logp = np.zeros((B, NCLS), np.float32)
    for core in range(NCORES):
        o1 = res[core]["out1"].reshape(MNB, S, NPOOL)      # [o, s, p]
        feats[core * S:(core + 1) * S] = o1.transpose(1, 0, 2).reshape(S, MNB * NPOOL)
        logp[core * S:(core + 1) * S] = res[core]["out2"]
    return feats, logp


def _install_ntff_hook():
    """Recreate antenv.axon_hooks + the ctypes NTFF hook (absent in this image)."""
    import types, ctypes, contextlib
    if "antenv.axon_hooks" in sys.modules:
        return
    so_path = "/opt/axon/libaxon_pjrt.so"
    lib = ctypes.CDLL(so_path)
    if not hasattr(lib, "axon_start_nrt_profile"):
        hook = None
    else:
        lib.axon_start_nrt_profile.argtypes = [ctypes.POINTER(ctypes.c_int64), ctypes.c_size_t]
        lib.axon_start_nrt_profile.restype = ctypes.c_int64
        lib.axon_stop_nrt_profile.argtypes = [ctypes.c_char_p]
        lib.axon_stop_nrt_profile.restype = ctypes.c_int64

        @contextlib.contextmanager
        def hook(output_dir, device_ids):
            import jax
            jax.devices()
            if device_ids:
                ids = (ctypes.c_int64 * len(device_ids))(*device_ids)
                rc = lib.axon_start_nrt_profile(ids, len(device_ids))
            else:
                rc = lib.axon_start_nrt_profile(None, 0)
            if rc != 0:
                raise RuntimeError(f"axon_start_nrt_profile rc={rc}")
            try:
                yield
            finally:
                n = lib.axon_stop_nrt_profile(str(output_dir).encode())
                print(f"profile: {n} file(s) written to {output_dir}", file=sys.stderr)

    mod = types.ModuleType("antenv.axon_hooks")
    mod.get_axon_ntff_profile_hook = lambda: hook
    mod.set_axon_ntff_profile_hook = lambda h: None
    sys.modules["antenv.axon_hooks"] = mod


def profile_run(**inputs):
    """Run with NTFF tracing; returns (exec_time_ns, profile info)."""
    _install_ntff_hook()
    x = np.asarray(inputs["x"], np.float32)
    consts = build_consts(inputs)
    nc = _get_nc()
    in_maps = []
    for core in range(NCORES):
        m = dict(consts)
        m["xtw"] = build_xtw(x[core * S:(core + 1) * S])
        in_maps.append(m)
    import tempfile
    tmpd = tempfile.mkdtemp(prefix="bassprof_")
    r = run_bass_kernel_spmd(nc, in_maps, list(range(NCORES)), trace=True, tmpdir=tmpd)
    return r.exec_time_ns, tmpd


# revision 28
# speedup vs baseline: 1.0379x; 1.0379x over previous
"""Trainium2 Bass kernel for nn_FBResEEGMoE (FIR filterbank + depthwise convs +
BN/ReLU + sconv + MoE gate/experts + FC + log_softmax).

Strategy (pure data parallelism, 8 cores x 32 samples):
  * The whole chain x -> FIR -> {conv3,conv5,conv7} -> concat/stride3-mean -> BN
    is linear per (band, channel): build the exact banded map L[nb] (1000x1000)
    on the host from the weights.
  * relu(z) = (z + |z|)/2, so pooled-sconv output
        hpool[nb,p,s] = sum_c (w[nb,c]/250) * sum_{t' in pool} (z + |z|)
    - sum z   : composed into tiny "PL" matmuls (linear path)
    - sum |z| : ONE pass over the big intermediate, done by DVE
                tensor_reduce(abs) directly from PSUM (fused ReLU+pool)
  * Big matmuls in fp16 (1 cyc/col on PE, fp32 PSUM accumulate); everything
    downstream of the pooling (gate softmax/top-3, experts, mixing, FC,
    log_softmax) is tiny and stays fp32 on-chip.

Outputs per core: out1 = mixed expert output [48(o), 8(p), 32(s)] and
out2 = logp [32(s), 9]; host reassembles (feats, logp) for the full batch.
"""

import sys

for _p in ("/opt/trn_rl_repo",):
    if _p not in sys.path:
        sys.path.insert(0, _p)

import numpy as np

import concourse.bass as bass
import concourse.bacc as bacc
import concourse.tile as tile
import concourse.mybir as mybir
from concourse.bass_utils import run_bass_kernel_spmd

F16 = mybir.dt.float16
F32 = mybir.dt.float32
AX = mybir.AxisListType
ALU = mybir.AluOpType
ACTF = mybir.ActivationFunctionType

# problem constants
B, C, T = 256, 22, 1000
NB, E, K = 6, 6, 3
POOL, NPOOL = 125, 8
MNB, NCLS = 48, 9
NUM_TAPS = 21
EPS = 1e-5

NCORES, S = 8, 32
CS = C * S                       # 704 flattened (c,s), col = c*S + s
CHUNKS = [(i * 128, min(128, CS - i * 128)) for i in range((CS + 127) // 128)]  # 6 chunks
NCH = len(CHUNKS)


# ----------------------------------------------------------------------------
# geometry (value-independent: derived from conv structure only)
# ----------------------------------------------------------------------------
def _support(tp):
    """Exact tau-support [lo, hi] of output t' (union over bands)."""
    taus = set()
    for i in (3 * tp, 3 * tp + 1, 3 * tp + 2):
        r, pos = i // T, i % T
        k = (3, 5, 7)[r]
        for t in range(max(0, pos - k // 2), min(T - 1, pos + k // 2) + 1):
            taus.add(max(0, t - 10))
            taus.add(min(T - 1, t + 10))
    return min(taus), max(taus)


RL = 28                     # t'-slots per (window, region) run
TLW = 3 * RL                # 84 column-slots per band per window
WROWS = 112                 # tau rows per window (+1 ones row)
ACT_FRAC = 0.55             # fraction of windows evacuated via ACT Abs lane


class Geometry:
    """Adaptive window placement + run/slot assignment for the abs-reduce."""

    def __init__(self):
        lohi = [_support(tp) for tp in range(T)]
        self.specials = [tp for tp in range(T) if lohi[tp][1] - lohi[tp][0] + 1 > WROWS]
        assert self.specials == [333, 666], self.specials
        sset = set(self.specials)
        region = {}
        reg_t = [[], [], []]
        for tp in range(T):
            if tp in sset:
                continue
            r = (3 * tp) // T
            region[tp] = r
            reg_t[r].append(tp)
        # adaptive window placement
        self.tau0 = []
        self.runs = []            # runs[w][r] = list of t'
        ptr = [0, 0, 0]
        while any(ptr[r] < len(reg_t[r]) for r in range(3)):
            cand = [lohi[reg_t[r][ptr[r]]][0] for r in range(3) if ptr[r] < len(reg_t[r])]
            t0 = min(cand)
            wrun = []
            for r in range(3):
                run = []
                while (ptr[r] < len(reg_t[r]) and len(run) < RL):
                    tp = reg_t[r][ptr[r]]
                    lo, hi = lohi[tp]
                    if lo >= t0 and hi <= t0 + WROWS - 1:
                        run.append(tp)
                        ptr[r] += 1
                    else:
                        break
                wrun.append(run)
            assert any(wrun), f"no progress at tau0={t0}"
            self.tau0.append(t0)
            self.runs.append(wrun)
        self.NW = len(self.tau0)
        assert all(region[tp] is not None for tp in region)
        n_assigned = sum(len(run) for wr in self.runs for run in wr)
        assert n_assigned == T - len(self.specials), n_assigned
        # slot assignment: for each window solve arithmetic-progression slots
        # for the 3 run-sums; tails (pool-crossing) get free slots.
        # Iterate SL until a consistent assignment exists.
        for SL in range(4, 40):
            ok = self._assign(SL)
            if ok:
                break
        assert ok, "no slot assignment found"
        self.SL = SL

    def _assign(self, SL):
        taken = [set() for _ in range(NPOOL)]
        self.call1 = []              # per w: (s0, D)
        self.tails = []              # per w: list of (r, ci, poolB, slotB, poolA, slotC)

        def alloc(pool, slot=None):
            if slot is None:
                cands = [s for s in range(SL) if s not in taken[pool]]
                if not cands:
                    return None
                slot = cands[0]
            elif slot in taken[pool]:
                return None
            taken[pool].add(slot)
            return slot

        for w in range(self.NW):
            wr = self.runs[w]
            pools = [wr[r][0] // POOL if wr[r] else None for r in range(3)]
            sol = None
            for slot0 in range(SL):
                p0 = pools[0] if pools[0] is not None else 0
                if slot0 in taken[p0]:
                    continue
                s0 = p0 * SL + slot0
                for D in range(1, 3 * SL + 1):
                    cols = [s0 + r * D for r in range(3)]
                    if cols[2] >= NPOOL * SL:
                        break
                    good = True
                    for r in range(3):
                        pr, sr = divmod(cols[r], SL)
                        if pools[r] is not None and pr != pools[r]:
                            good = False
                            break
                        if sr in taken[pr]:
                            good = False
                            break
                    if good:
                        sol = (s0, D, cols)
                        break
                if sol:
                    break
            if sol is None:
                return False
            s0, D, cols = sol
            for c in cols:
                taken[c // SL].add(c % SL)
            self.call1.append((s0, D))
            wt = []
            for r in range(3):
                run = self.runs[w][r]
                if not run:
                    continue
                pA = run[0] // POOL
                ci = None
                for j, tp in enumerate(run):
                    if tp // POOL != pA:
                        ci = j
                        break
                if ci is not None:
                    pB = run[ci] // POOL
                    sB = alloc(pB)
                    sC = alloc(pA)
                    if sB is None or sC is None:
                        return False
                    wt.append((r, ci, pB, sB, pA, sC))
            self.tails.append(wt)
        self.sp_slot = {}
        for pool_id in (2, 5):
            s = alloc(pool_id)
            if s is None:
                return False
            self.sp_slot[pool_id] = s
        return True


GEO = Geometry()
SL = GEO.SL
NW = GEO.NW


# ----------------------------------------------------------------------------
# host-side constant construction (depends on weight values)
# ----------------------------------------------------------------------------
def _build_L(fir, w3, w5, w7, scale1):
    """L2[nb] (T x T): BN-scaled pre-ReLU map, h_pre[nb,c,t'] = L2[nb] @ x[c]."""
    idx = np.arange(T)
    Ls = np.zeros((NB, T, T))
    for nb in range(NB):
        F = np.zeros((T, T))
        for k in range(NUM_TAPS):
            tau = idx + k - 10
            v = (tau >= 0) & (tau < T)
            F[idx[v], tau[v]] += float(fir[nb, k])

        def convF(wv):
            kk = len(wv)
            Ck = np.zeros_like(F)
            for j in range(kk):
                src = idx + j - kk // 2
                v = (src >= 0) & (src < T)
                Ck[idx[v]] += float(wv[j]) * F[src[v]]
            return Ck

        CONCAT = np.concatenate([convF(w3[nb]), convF(w5[nb]), convF(w7[nb])], axis=0)
        Ls[nb] = (CONCAT[3 * idx] + CONCAT[3 * idx + 1] + CONCAT[3 * idx + 2]) / 3.0
    return Ls * scale1[:, None, None]


def build_consts(inputs):
    """All device input arrays (shared across cores) from the raw weights."""
    fir = np.asarray(inputs["fir_kernels"], np.float64)
    w3 = np.asarray(inputs["pconv_w3"], np.float64)[:, 0, 0, :]
    w5 = np.asarray(inputs["pconv_w5"], np.float64)[:, 0, 0, :]
    w7 = np.asarray(inputs["pconv_w7"], np.float64)[:, 0, 0, :]
    scale1 = np.asarray(inputs["bn1_g"], np.float64) / np.sqrt(np.asarray(inputs["bn1_v"], np.float64) + EPS)
    beta1 = np.asarray(inputs["bn1_b"], np.float64) - np.asarray(inputs["bn1_m"], np.float64) * scale1
    sconv = np.asarray(inputs["sconv_w"], np.float64)[:, 0, :, 0]     # (NB, C)
    gate_w = np.asarray(inputs["gate_w"], np.float64)
    gate_b = np.asarray(inputs["gate_b"], np.float64)
    exp_w = np.asarray(inputs["exp_w"], np.float64)
    exp_b = np.asarray(inputs["exp_b"], np.float64)
    escale = np.asarray(inputs["ebn_g"], np.float64) / np.sqrt(np.asarray(inputs["ebn_v"], np.float64) + EPS)
    ebeta = np.asarray(inputs["ebn_b"], np.float64) - np.asarray(inputs["ebn_m"], np.float64) * escale
    fc_w = np.asarray(inputs["fc_w"], np.float64)
    fc_b = np.asarray(inputs["fc_b"], np.float64)

    L2 = _build_L(fir, w3, w5, w7, scale1)
    bh = beta1                                            # bias of z per band

    # R blocks: [128(row), NW, 6(nb), 3(run), RL]
    R = np.zeros((128, NW, NB, 3, RL), np.float32)
    for w in range(NW):
        t0 = GEO.tau0[w]
        hi = min(T, t0 + WROWS)
        for nb in range(NB):
            for r in range(3):
                run = GEO.runs[w][r]
                if not run:
                    continue
                R[0, w, nb, r, :len(run)] = bh[nb]
                R[1:1 + hi - t0, w, nb, r, :len(run)] = L2[nb, run, t0:hi].T
    # specials: [128, 2(part), 6(nb), 2(which)]
    RS = np.zeros((128, 2, NB, 2), np.float32)
    tlast = GEO.tau0[NW - 1]
    for si, tp in enumerate(GEO.specials):
        for nb in range(NB):
            RS[0, 0, nb, si] = bh[nb]
            RS[1:1 + WROWS, 0, nb, si] = L2[nb, tp, 0:WROWS]
            RS[1:1 + T - tlast, 1, nb, si] = L2[nb, tp, tlast:T]
            assert np.all(L2[nb, tp, WROWS:tlast] == 0.0)

    # PL blocks: [128, NW, 48] (pool-summed rows incl. bias; specials folded)
    PLw = np.zeros((128, NW, MNB), np.float64)
    for w in range(NW):
        t0 = GEO.tau0[w]
        hi = min(T, t0 + WROWS)
        for nb in range(NB):
            for r in range(3):
                for tp in GEO.runs[w][r]:
                    col = nb * NPOOL + tp // POOL
                    PLw[0, w, col] += bh[nb]
                    PLw[1:1 + hi - t0, w, col] += L2[nb, tp, t0:hi]
    for si, tp in enumerate(GEO.specials):
        for nb in range(NB):
            col = nb * NPOOL + tp // POOL
            PLw[0, 0, col] += bh[nb]
            PLw[1:1 + WROWS, 0, col] += L2[nb, tp, 0:WROWS]
            PLw[1:1 + T - tlast, NW - 1, col] += L2[nb, tp, tlast:T]

    # Wpat: [128, NCH, 2, 96] fp32 : delta(s,s') * sconv[3h+nbr, c]/250
    Wp = np.zeros((128, NCH, 2, 96), np.float32)
    for ch, (cs0, P) in enumerate(CHUNKS):
        for p in range(P):
            c, s = divmod(cs0 + p, S)
            for h in range(2):
                for nbr in range(3):
                    Wp[p, ch, h, nbr * S + s] = sconv[3 * h + nbr, c] / 250.0

    gateW = np.zeros((7, MNB), np.float32)
    gateW[:6] = (gate_w.T / NPOOL)
    gateW[6] = gate_b

    expW = np.zeros((7, 3, 96), np.float32)
    for ch3 in range(3):
        for j in range(96):
            eo = ch3 * 96 + j
            e, o = divmod(eo, MNB)
            expW[:6, ch3, j] = exp_w[e, o, :] * escale[e, o]
            expW[6, ch3, j] = exp_b[e, o] * escale[e, o] + ebeta[e, o]

    SEL = np.zeros((96, 3, MNB), np.float32)
    for ch3 in range(3):
        for j in range(96):
            SEL[j, ch3, (ch3 * 96 + j) % MNB] = 1.0

    fcW = np.zeros((49, NPOOL, NCLS), np.float32)
    for p in range(NPOOL):
        for o in range(MNB):
            fcW[o, p, :] = fc_w[:, o * NPOOL + p]
    fcW[48, 0, :] = fc_b

    iden = np.eye(32, dtype=np.float32)

    return {
        "rmat": R.reshape(128, -1).astype(np.float16),
        "rspec": RS.reshape(128, -1).astype(np.float16),
        "plmat": PLw.reshape(128, -1).astype(np.float16),
        "wpat": Wp.reshape(128, -1),
        "gatew": gateW,
        "expw": expW.reshape(7, -1),
        "selm": SEL.reshape(96, -1),
        "fcw": fcW.reshape(49, -1),
        "iden": iden,
        "onesrow": np.ones((1, NPOOL * S), np.float32),
    }


def build_xtw(xcore):
    """Per-core x windows: [128, NW, CS] fp16 (row 0 = ones, rows 1.. = x^T)."""
    xt = np.zeros((128, NW, CS), np.float16)
    xt[0] = 1.0
    xpad = np.zeros((S, C, GEO.tau0[-1] + WROWS), np.float32)
    xpad[:, :, :T] = xcore
    for w in range(NW):
        t0 = GEO.tau0[w]
        blk = xpad[:, :, t0:t0 + WROWS]                   # (S, C, 112)
        xt[1:1 + WROWS, w, :] = blk.transpose(2, 1, 0).reshape(WROWS, CS).astype(np.float16)
    return xt.reshape(128, -1)


# ----------------------------------------------------------------------------
# bass program
# ----------------------------------------------------------------------------

def build_program():
    nc = bacc.Bacc("TRN2", target_bir_lowering=False, debug=False)

    din = {}
    for name, shape, dt in [
        ("xtw", (128, NW * CS), F16),
        ("rmat", (128, NW * NB * 3 * RL), F16),
        ("rspec", (128, 2 * NB * 2), F16),
        ("plmat", (128, NW * MNB), F16),
        ("wpat", (128, NCH * 2 * 96), F32),
        ("gatew", (7, MNB), F32),
        ("expw", (7, 3 * 96), F32),
        ("selm", (96, 3 * MNB), F32),
        ("fcw", (49, NPOOL * NCLS), F32),
        ("iden", (32, 32), F32),
        ("onesrow", (1, NPOOL * S), F32),
    ]:
        din[name] = nc.dram_tensor(name, list(shape), dt, kind="ExternalInput").ap()
    g6d = nc.dram_tensor("g6scratch", [E, S], F32, kind="Internal").ap()
    out1 = nc.dram_tensor("out1", [MNB, NPOOL * S], F32, kind="ExternalOutput").ap()
    out2 = nc.dram_tensor("out2", [S, NCLS], F32, kind="ExternalOutput").ap()

    with tile.TileContext(nc) as tc:
        with (
            tc.tile_pool(name="consts", bufs=1) as cpool,
            tc.tile_pool(name="ps", bufs=3, space="PSUM") as ps_pool,
            tc.tile_pool(name="acc", bufs=1, space="PSUM") as acc_pool,
            tc.tile_pool(name="cc", bufs=1, space="PSUM") as cc_pool,
            tc.tile_pool(name="sb", bufs=2) as sbp,
            tc.tile_pool(name="small", bufs=1) as sm,
        ):
            # ---- load constants ----
            sb = {}
            for name, ap in din.items():
                t = cpool.tile(list(ap.shape), ap.dtype, tag=name)
                nc.sync.dma_start(t[:], ap)
                sb[name] = t
            xtw = sb["xtw"][:].rearrange("p (w n) -> p w n", w=NW)         # [128, NW, 704]
            rmat = sb["rmat"][:].rearrange("p (w n) -> p w n", w=NW)         # [128, NW, 504]
            plm = sb["plmat"][:].rearrange("p (w n) -> p w n", w=NW)
            rsp = sb["rspec"][:].rearrange("p (k n w2) -> p k n w2", k=2, n=NB)
            wpat = sb["wpat"][:].rearrange("p (c h m) -> p c h m", c=NCH, h=2)
            expw = sb["expw"][:].rearrange("p (c m) -> p c m", c=3)
            selm = sb["selm"][:].rearrange("p (c m) -> p c m", c=3)
            fcw = sb["fcw"][:].rearrange("p (k n) -> p k n", k=NPOOL)

            cc_ps = [cc_pool.tile([96, MNB], F32, tag=f"cc{h}", name=f"cc{h}") for h in range(2)]

            # ================= main loop: chunks x windows =================
            for ch, (cs0, P) in enumerate(CHUNKS):
                lin_ps = acc_pool.tile([P, MNB], F32, tag="lin")
                sp_ps = acc_pool.tile([P, NB * 2], F32, tag="sp")
                partials = sbp.tile([128, MNB * SL], F32, tag="partials")
                nc.vector.memset(partials[:], 0.0)
                pview = partials[:].rearrange("p (n sl) -> p n sl", sl=SL)

                for w in range(NW):
                    lhsT = xtw[:, w, cs0:cs0 + P]
                    nc.tensor.matmul(lin_ps[:], lhsT, plm[:, w, :],
                                     start=(w == 0), stop=(w == NW - 1))
                    if w == 0:
                        nc.tensor.matmul(sp_ps[:], lhsT,
                                         rsp[:, 0].rearrange("p n w2 -> p (n w2)"),
                                         start=True, stop=False)
                    if w == NW - 1:
                        nc.tensor.matmul(sp_ps[:], lhsT,
                                         rsp[:, 1].rearrange("p n w2 -> p (n w2)"),
                                         start=False, stop=True)
                    pst = ps_pool.tile([P, NB * TLW], F32, tag="ps", name="pst")
                    nc.tensor.matmul(pst[:], lhsT, rmat[:, w, :], start=True, stop=True)
                    pg = partials[:P].rearrange("p (n x) -> p n x", n=NB)
                    pv = pst[:].rearrange("p (n r j) -> p n r j", n=NB, r=3)
                    s0, D = GEO.call1[w]
                    nc.vector.tensor_reduce(
                        pg[:, :, s0:s0 + 2 * D + 1:D], pv,
                        axis=AX.X, op=ALU.add, apply_absolute_value=True)
                    # tails: pool-crossing corrections (always fp32 PSUM source)
                    for (r, ci, pB, sB, pA, sC) in GEO.tails[w]:
                        tin = pv[:, :, r, ci:RL]
                        nc.vector.tensor_reduce(
                            pg[:, :, pB * SL + sB:pB * SL + sB + 1], tin,
                            axis=AX.X, op=ALU.add, apply_absolute_value=True)
                        nc.vector.tensor_reduce(
                            pg[:, :, pA * SL + sC:pA * SL + sC + 1], tin,
                            axis=AX.X, op=ALU.add, apply_absolute_value=True,
                            negate=True)
                # specials reduce (which=0 -> pool2, which=1 -> pool5)
                for si, pool_id in ((0, 2), (1, 5)):
                    col = pool_id * SL + GEO.sp_slot[pool_id]
                    nc.vector.tensor_reduce(
                        partials[:P].rearrange("p (n x) -> p n x", n=NB)[:, :, col:col + 1],
                        sp_ps[:].rearrange("p (n w2) -> p n w2", w2=2)[:, :, si:si + 1],
                        axis=AX.X, op=ALU.add, apply_absolute_value=True)
                # combine: hpS = sum_slots partials + lin
                hpS = sbp.tile([P, MNB], F32, tag="hpS")
                nc.vector.tensor_reduce(hpS[:], pview[:P], axis=AX.X, op=ALU.add)
                nc.vector.tensor_tensor(hpS[:], hpS[:], lin_ps[:], ALU.add)
                for h in range(2):
                    nc.tensor.matmul(cc_ps[h][:], wpat[:P, ch, h, :], hpS[:],
                                     start=(ch == 0), stop=(ch == NCH - 1))

            # ========================= head =========================
            ccS = [sm.tile([96, MNB], F32, tag=f"ccS{h}", name=f"ccS{h}") for h in range(2)]
            for h in range(2):
                nc.vector.tensor_copy(ccS[h][:], cc_ps[h][:])
            hpT = sm.tile([7, NPOOL * S], F32, tag="hpT")   # rows 0-5 hpoolT, row6 ones
            nc.sync.dma_start(hpT[6:7, :], din["onesrow"])
            for h in range(2):
                for nbr in range(3):
                    nb = 3 * h + nbr
                    src = ccS[h][nbr * S:(nbr + 1) * S, nb * NPOOL:(nb + 1) * NPOOL]  # [32(s), 8(p)]
                    dst = hpT[nb:nb + 1, :].rearrange("one (s p) -> one s p", s=S)
                    nc.sync.dma_start(dst, src)
            # gate tile [7, 32]: rows 0-5 = sum_p hpoolT, row 6 ones
            gt = sm.tile([7, S], F32, tag="gt")
            nc.sync.dma_start(gt[6:7, :], din["onesrow"][:, 0:S])
            nc.vector.tensor_reduce(
                gt[0:6, :], hpT[0:6, :].rearrange("n (s p) -> n s p", s=S),
                axis=AX.X, op=ALU.add)
            glog = ps_pool.tile([S, MNB], F32, tag="ps")
            nc.tensor.matmul(glog[:], gt[:], sb["gatew"][:], start=True, stop=True)
            # softmax
            m1 = sm.tile([S, 1], F32, tag="m1")
            nc.vector.tensor_reduce(m1[:], glog[:], axis=AX.X, op=ALU.max)
            nm1 = sm.tile([S, 1], F32, tag="nm1")
            nc.vector.tensor_scalar_mul(nm1[:], m1[:], -1.0)
            ex = sm.tile([S, MNB], F32, tag="ex")
            nc.scalar.activation(ex[:], glog[:], ACTF.Exp, bias=nm1[:], scale=1.0)
            sm1 = sm.tile([S, 1], F32, tag="sm1")
            nc.vector.tensor_reduce(sm1[:], ex[:], axis=AX.X, op=ALU.add)
            rs1 = sm.tile([S, 1], F32, tag="rs1")
            nc.vector.reciprocal(rs1[:], sm1[:])
            gw = sm.tile([S, MNB], F32, tag="gw")
            nc.vector.tensor_scalar(gw[:], ex[:], rs1[:], None, op0=ALU.mult)
            # top-3 mask
            msum = sm.tile([S, MNB], F32, tag="msum")
            nc.vector.memset(msum[:], 0.0)
            cur = gw
            for it in range(K):
                mx = sm.tile([S, 1], F32, tag=f"mx{it}")
                nc.vector.tensor_reduce(mx[:], cur[:], axis=AX.X, op=ALU.max)
                mk = sm.tile([S, MNB], F32, tag=f"mk{it}")
                nc.vector.tensor_scalar(mk[:], cur[:], mx[:], None, op0=ALU.is_ge)
                nc.vector.tensor_tensor(msum[:], msum[:], mk[:], ALU.add)
                if it < K - 1:
                    nxt = sm.tile([S, MNB], F32, tag=f"cur{it}")
                    big = sm.tile([S, MNB], F32, tag=f"big{it}")
                    nc.vector.tensor_scalar_mul(big[:], mk[:], 1e30)
                    nc.vector.tensor_tensor(nxt[:], cur[:], big[:], ALU.subtract)
                    cur = nxt
            gwm = sm.tile([S, MNB], F32, tag="gwm")
            nc.vector.tensor_tensor(gwm[:], gw[:], msum[:], ALU.mult)
            den = sm.tile([S, 1], F32, tag="den")
            nc.vector.tensor_reduce(den[:], gwm[:], axis=AX.X, op=ALU.add)
            rden = sm.tile([S, 1], F32, tag="rden")
            nc.vector.reciprocal(rden[:], den[:])
            gwn = sm.tile([S, MNB], F32, tag="gwn")
            nc.vector.tensor_scalar(gwn[:], gwm[:], rden[:], None, op0=ALU.mult)
            # gw6T [6, 32] via PE transpose
            g6ps = ps_pool.tile([E, S], F32, tag="ps")
            nc.tensor.matmul(g6ps[:], gwn[:, 0:E], sb["iden"][:], is_transpose=True)
            gw6T = sm.tile([E, S], F32, tag="gw6T")
            nc.vector.tensor_copy(gw6T[:], g6ps[:])
            nc.sync.dma_start(g6d, gw6T[:])
            # experts + mixing
            mix_ps = ps_pool.tile([MNB, NPOOL * S], F32, tag="ps")
            for ch3 in range(3):
                yp_ps = ps_pool.tile([96, NPOOL * S], F32, tag="ps")
                nc.tensor.matmul(yp_ps[:], expw[:, ch3, :], hpT[:], start=True, stop=True)
                ypR = sbp.tile([96, NPOOL * S], F32, tag="ypR")
                nc.scalar.activation(ypR[:], yp_ps[:], ACTF.Relu)
                G = sbp.tile([96, NPOOL * S], F32, tag="G")
                for er in range(2):
                    e = 2 * ch3 + er
                    # G stored (p,s)-major: innermost s is contiguous on both sides
                    srcb = g6d[e:e + 1, :].unsqueeze(1).broadcast_to([48, NPOOL, S])
                    dstb = G[er * 48:(er + 1) * 48, :].rearrange("o (p s) -> o p s", p=NPOOL)
                    nc.sync.dma_start(dstb, srcb)
                Gm = sbp.tile([96, NPOOL * S], F32, tag="Gm")
                ypv = ypR[:].rearrange("o (s p) -> o s p", s=S)
                Gv = G[:].rearrange("o (p s) -> o p s", p=NPOOL).transpose([0, 2, 1])
                Gmv = Gm[:].rearrange("o (s p) -> o s p", s=S)
                nc.vector.tensor_tensor(Gmv, ypv, Gv, ALU.mult)
                nc.tensor.matmul(mix_ps[:], selm[:, ch3, :], Gm[:],
                                 start=(ch3 == 0), stop=(ch3 == 2))
            mixS = sm.tile([49, NPOOL * S], F32, tag="mixS")
            nc.sync.dma_start(mixS[48:49, :], din["onesrow"])
            nc.vector.tensor_copy(mixS[0:48, :], mix_ps[:])
            nc.sync.dma_start(out1, mixS[0:48, :])
            # fc
            lg_ps = ps_pool.tile([S, NCLS], F32, tag="ps")
            mixv = mixS[:].rearrange("k (s p) -> k s p", s=S)
            for p in range(NPOOL):
                nc.tensor.matmul(lg_ps[:], mixv[:, :, p], fcw[:, p, :],
                                 start=(p == 0), stop=(p == NPOOL - 1))
            m2 = sm.tile([S, 1], F32, tag="m2")
            nc.vector.tensor_reduce(m2[:], lg_ps[:], axis=AX.X, op=ALU.max)
            nm2 = sm.tile([S, 1], F32, tag="nm2")
            nc.vector.tensor_scalar_mul(nm2[:], m2[:], -1.0)
            ex2 = sm.tile([S, NCLS], F32, tag="ex2")
            nc.scalar.activation(ex2[:], lg_ps[:], ACTF.Exp, bias=nm2[:], scale=1.0)
            s2 = sm.tile([S, 1], F32, tag="s2")
            nc.vector.tensor_reduce(s2[:], ex2[:], axis=AX.X, op=ALU.add)
            ls2 = sm.tile([S, 1], F32, tag="ls2")
            nc.scalar.activation(ls2[:], s2[:], ACTF.Ln)
            lgp = sm.tile([S, NCLS], F32, tag="lgp")
            nc.vector.tensor_scalar(lgp[:], lg_ps[:], m2[:], ls2[:],
                                    op0=ALU.subtract, op1=ALU.subtract)
            nc.sync.dma_start(out2, lgp[:])

    nc.compile()
    return nc


_NC = None


def _get_nc():
    global _NC
    if _NC is None:
        _NC = build_program()
    return _NC


def kernel(**inputs):
    x = np.asarray(inputs["x"], np.float32)
    consts = build_consts(inputs)
    nc = _get_nc()
    in_maps = []
    for core in range(NCORES):
        m = dict(consts)
        m["xtw"] = build_xtw(x[core * S:(core + 1) * S])
        in_maps.append(m)
    res = run_bass_kernel_spmd(nc, in_maps, list(range(NCORES))).results
    feats = np.zeros((B, MNB * NPOOL), np.float32)
    logp = np.zeros((B, NCLS), np.float32)
    for core in range(NCORES):
        o1 = res[core]["out1"].reshape(MNB, S, NPOOL)      # [o, s, p]
        feats[core * S:(core + 1) * S] = o1.transpose(1, 0, 2).reshape(S, MNB * NPOOL)
        logp[core * S:(core + 1) * S] = res[core]["out2"]
    return feats, logp


def _install_ntff_hook():
    """Recreate antenv.axon_hooks + the ctypes NTFF hook (absent in this image)."""
    import types, ctypes, contextlib
    if "antenv.axon_hooks" in sys.modules:
        return
    so_path = "/opt/axon/libaxon_pjrt.so"
    lib = ctypes.CDLL(so_path)
    if not hasattr(lib, "axon_start_nrt_profile"):
        hook = None
    else:
        lib.axon_start_nrt_profile.argtypes = [ctypes.POINTER(ctypes.c_int64), ctypes.c_size_t]
        lib.axon_start_nrt_profile.restype = ctypes.c_int64
        lib.axon_stop_nrt_profile.argtypes = [ctypes.c_char_p]
        lib.axon_stop_nrt_profile.restype = ctypes.c_int64

        @contextlib.contextmanager
        def hook(output_dir, device_ids):
            import jax
            jax.devices()
            if device_ids:
                ids = (ctypes.c_int64 * len(device_ids))(*device_ids)
                rc = lib.axon_start_nrt_profile(ids, len(device_ids))
            else:
                rc = lib.axon_start_nrt_profile(None, 0)
            if rc != 0:
                raise RuntimeError(f"axon_start_nrt_profile rc={rc}")
            try:
                yield
            finally:
                n = lib.axon_stop_nrt_profile(str(output_dir).encode())
                print(f"profile: {n} file(s) written to {output_dir}", file=sys.stderr)

    mod = types.ModuleType("antenv.axon_hooks")
    mod.get_axon_ntff_profile_hook = lambda: hook
    mod.set_axon_ntff_profile_hook = lambda h: None
    sys.modules["antenv.axon_hooks"] = mod


def profile_run(**inputs):
    """Run with NTFF tracing; returns (exec_time_ns, profile info)."""
    _install_ntff_hook()
    x = np.asarray(inputs["x"], np.float32)
    consts = build_consts(inputs)
    nc = _get_nc()
    in_maps = []
    for core in range(NCORES):
        m = dict(consts)
        m["xtw"] = build_xtw(x[core * S:(core + 1) * S])
        in_maps.append(m)
    import tempfile
    tmpd = tempfile.mkdtemp(prefix="bassprof_")
    r = run_bass_kernel_spmd(nc, in_maps, list(range(NCORES)), trace=True, tmpdir=tmpd)
    return r.exec_time_ns, tmpd


# revision 29
# speedup vs baseline: 1.0822x; 1.0427x over previous
"""Trainium2 Bass kernel for nn_FBResEEGMoE (FIR filterbank + depthwise convs +
BN/ReLU + sconv + MoE gate/experts + FC + log_softmax).

Strategy (pure data parallelism, 8 cores x 32 samples):
  * The whole chain x -> FIR -> {conv3,conv5,conv7} -> concat/stride3-mean -> BN
    is linear per (band, channel): build the exact banded map L[nb] (1000x1000)
    on the host from the weights.
  * relu(z) = (z + |z|)/2, so pooled-sconv output
        hpool[nb,p,s] = sum_c (w[nb,c]/250) * sum_{t' in pool} (z + |z|)
    - sum z   : composed into tiny "PL" matmuls (linear path)
    - sum |z| : ONE pass over the big intermediate, done by DVE
                tensor_reduce(abs) directly from PSUM (fused ReLU+pool)
  * Big matmuls in fp16 (1 cyc/col on PE, fp32 PSUM accumulate); everything
    downstream of the pooling (gate softmax/top-3, experts, mixing, FC,
    log_softmax) is tiny and stays fp32 on-chip.

Outputs per core: out1 = mixed expert output [48(o), 8(p), 32(s)] and
out2 = logp [32(s), 9]; host reassembles (feats, logp) for the full batch.
"""

import sys

for _p in ("/opt/trn_rl_repo",):
    if _p not in sys.path:
        sys.path.insert(0, _p)

import numpy as np

import concourse.bass as bass
import concourse.bacc as bacc
import concourse.tile as tile
import concourse.mybir as mybir
from concourse.bass_utils import run_bass_kernel_spmd

F16 = mybir.dt.float16
F32 = mybir.dt.float32
AX = mybir.AxisListType
ALU = mybir.AluOpType
ACTF = mybir.ActivationFunctionType

# problem constants
B, C, T = 256, 22, 1000
NB, E, K = 6, 6, 3
POOL, NPOOL = 125, 8
MNB, NCLS = 48, 9
NUM_TAPS = 21
EPS = 1e-5

NCORES, S = 8, 32
CS = C * S                       # 704 flattened (c,s), col = c*S + s
CHUNKS = [(i * 128, min(128, CS - i * 128)) for i in range((CS + 127) // 128)]  # 6 chunks
NCH = len(CHUNKS)


# ----------------------------------------------------------------------------
# geometry (value-independent: derived from conv structure only)
# ----------------------------------------------------------------------------
def _support(tp):
    """Exact tau-support [lo, hi] of output t' (union over bands)."""
    taus = set()
    for i in (3 * tp, 3 * tp + 1, 3 * tp + 2):
        r, pos = i // T, i % T
        k = (3, 5, 7)[r]
        for t in range(max(0, pos - k // 2), min(T - 1, pos + k // 2) + 1):
            taus.add(max(0, t - 10))
            taus.add(min(T - 1, t + 10))
    return min(taus), max(taus)


RL = 28                     # t'-slots per (window, region) run
TLW = 3 * RL                # 84 column-slots per band per window
WROWS = 112                 # tau rows per window (+1 ones row)
ACT_FRAC = 0.55             # fraction of windows evacuated via ACT Abs lane


class Geometry:
    """Adaptive window placement + run/slot assignment for the abs-reduce."""

    def __init__(self):
        lohi = [_support(tp) for tp in range(T)]
        self.specials = [tp for tp in range(T) if lohi[tp][1] - lohi[tp][0] + 1 > WROWS]
        assert self.specials == [333, 666], self.specials
        sset = set(self.specials)
        region = {}
        reg_t = [[], [], []]
        for tp in range(T):
            if tp in sset:
                continue
            r = (3 * tp) // T
            region[tp] = r
            reg_t[r].append(tp)
        # adaptive window placement
        self.tau0 = []
        self.runs = []            # runs[w][r] = list of t'
        ptr = [0, 0, 0]
        while any(ptr[r] < len(reg_t[r]) for r in range(3)):
            cand = [lohi[reg_t[r][ptr[r]]][0] for r in range(3) if ptr[r] < len(reg_t[r])]
            t0 = min(cand)
            wrun = []
            for r in range(3):
                run = []
                while (ptr[r] < len(reg_t[r]) and len(run) < RL):
                    tp = reg_t[r][ptr[r]]
                    lo, hi = lohi[tp]
                    if lo >= t0 and hi <= t0 + WROWS - 1:
                        run.append(tp)
                        ptr[r] += 1
                    else:
                        break
                wrun.append(run)
            assert any(wrun), f"no progress at tau0={t0}"
            self.tau0.append(t0)
            self.runs.append(wrun)
        self.NW = len(self.tau0)
        assert all(region[tp] is not None for tp in region)
        n_assigned = sum(len(run) for wr in self.runs for run in wr)
        assert n_assigned == T - len(self.specials), n_assigned
        # slot assignment: for each window solve arithmetic-progression slots
        # for the 3 run-sums; tails (pool-crossing) get free slots.
        # Iterate SL until a consistent assignment exists.
        for SL in range(4, 40):
            ok = self._assign(SL)
            if ok:
                break
        assert ok, "no slot assignment found"
        self.SL = SL

    def _assign(self, SL):
        taken = [set() for _ in range(NPOOL)]
        self.call1 = []              # per w: (s0, D)
        self.tails = []              # per w: list of (r, ci, poolB, slotB, poolA, slotC)

        def alloc(pool, slot=None):
            if slot is None:
                cands = [s for s in range(SL) if s not in taken[pool]]
                if not cands:
                    return None
                slot = cands[0]
            elif slot in taken[pool]:
                return None
            taken[pool].add(slot)
            return slot

        for w in range(self.NW):
            wr = self.runs[w]
            pools = [wr[r][0] // POOL if wr[r] else None for r in range(3)]
            sol = None
            for slot0 in range(SL):
                p0 = pools[0] if pools[0] is not None else 0
                if slot0 in taken[p0]:
                    continue
                s0 = p0 * SL + slot0
                for D in range(1, 3 * SL + 1):
                    cols = [s0 + r * D for r in range(3)]
                    if cols[2] >= NPOOL * SL:
                        break
                    good = True
                    for r in range(3):
                        pr, sr = divmod(cols[r], SL)
                        if pools[r] is not None and pr != pools[r]:
                            good = False
                            break
                        if sr in taken[pr]:
                            good = False
                            break
                    if good:
                        sol = (s0, D, cols)
                        break
                if sol:
                    break
            if sol is None:
                return False
            s0, D, cols = sol
            for c in cols:
                taken[c // SL].add(c % SL)
            self.call1.append((s0, D))
            wt = []
            for r in range(3):
                run = self.runs[w][r]
                if not run:
                    continue
                pA = run[0] // POOL
                ci = None
                for j, tp in enumerate(run):
                    if tp // POOL != pA:
                        ci = j
                        break
                if ci is not None:
                    pB = run[ci] // POOL
                    sB = alloc(pB)
                    sC = alloc(pA)
                    if sB is None or sC is None:
                        return False
                    wt.append((r, ci, pB, sB, pA, sC))
            self.tails.append(wt)
        self.sp_slot = {}
        for pool_id in (2, 5):
            s = alloc(pool_id)
            if s is None:
                return False
            self.sp_slot[pool_id] = s
        return True


GEO = Geometry()
SL = GEO.SL
NW = GEO.NW


# ----------------------------------------------------------------------------
# host-side constant construction (depends on weight values)
# ----------------------------------------------------------------------------
def _build_L(fir, w3, w5, w7, scale1):
    """L2[nb] (T x T): BN-scaled pre-ReLU map, h_pre[nb,c,t'] = L2[nb] @ x[c]."""
    idx = np.arange(T)
    Ls = np.zeros((NB, T, T))
    for nb in range(NB):
        F = np.zeros((T, T))
        for k in range(NUM_TAPS):
            tau = idx + k - 10
            v = (tau >= 0) & (tau < T)
            F[idx[v], tau[v]] += float(fir[nb, k])

        def convF(wv):
            kk = len(wv)
            Ck = np.zeros_like(F)
            for j in range(kk):
                src = idx + j - kk // 2
                v = (src >= 0) & (src < T)
                Ck[idx[v]] += float(wv[j]) * F[src[v]]
            return Ck

        CONCAT = np.concatenate([convF(w3[nb]), convF(w5[nb]), convF(w7[nb])], axis=0)
        Ls[nb] = (CONCAT[3 * idx] + CONCAT[3 * idx + 1] + CONCAT[3 * idx + 2]) / 3.0
    return Ls * scale1[:, None, None]


def build_consts(inputs):
    """All device input arrays (shared across cores) from the raw weights."""
    fir = np.asarray(inputs["fir_kernels"], np.float64)
    w3 = np.asarray(inputs["pconv_w3"], np.float64)[:, 0, 0, :]
    w5 = np.asarray(inputs["pconv_w5"], np.float64)[:, 0, 0, :]
    w7 = np.asarray(inputs["pconv_w7"], np.float64)[:, 0, 0, :]
    scale1 = np.asarray(inputs["bn1_g"], np.float64) / np.sqrt(np.asarray(inputs["bn1_v"], np.float64) + EPS)
    beta1 = np.asarray(inputs["bn1_b"], np.float64) - np.asarray(inputs["bn1_m"], np.float64) * scale1
    sconv = np.asarray(inputs["sconv_w"], np.float64)[:, 0, :, 0]     # (NB, C)
    gate_w = np.asarray(inputs["gate_w"], np.float64)
    gate_b = np.asarray(inputs["gate_b"], np.float64)
    exp_w = np.asarray(inputs["exp_w"], np.float64)
    exp_b = np.asarray(inputs["exp_b"], np.float64)
    escale = np.asarray(inputs["ebn_g"], np.float64) / np.sqrt(np.asarray(inputs["ebn_v"], np.float64) + EPS)
    ebeta = np.asarray(inputs["ebn_b"], np.float64) - np.asarray(inputs["ebn_m"], np.float64) * escale
    fc_w = np.asarray(inputs["fc_w"], np.float64)
    fc_b = np.asarray(inputs["fc_b"], np.float64)

    L2 = _build_L(fir, w3, w5, w7, scale1)
    bh = beta1                                            # bias of z per band

    # R blocks: [128(row), NW, 6(nb), 3(run), RL]
    R = np.zeros((128, NW, NB, 3, RL), np.float32)
    for w in range(NW):
        t0 = GEO.tau0[w]
        hi = min(T, t0 + WROWS)
        for nb in range(NB):
            for r in range(3):
                run = GEO.runs[w][r]
                if not run:
                    continue
                R[0, w, nb, r, :len(run)] = bh[nb]
                R[1:1 + hi - t0, w, nb, r, :len(run)] = L2[nb, run, t0:hi].T
    # specials: [128, 2(part), 6(nb), 2(which)]
    RS = np.zeros((128, 2, NB, 2), np.float32)
    tlast = GEO.tau0[NW - 1]
    for si, tp in enumerate(GEO.specials):
        for nb in range(NB):
            RS[0, 0, nb, si] = bh[nb]
            RS[1:1 + WROWS, 0, nb, si] = L2[nb, tp, 0:WROWS]
            RS[1:1 + T - tlast, 1, nb, si] = L2[nb, tp, tlast:T]
            assert np.all(L2[nb, tp, WROWS:tlast] == 0.0)

    # PL blocks: [128, NW, 48] (pool-summed rows incl. bias; specials folded)
    PLw = np.zeros((128, NW, MNB), np.float64)
    for w in range(NW):
        t0 = GEO.tau0[w]
        hi = min(T, t0 + WROWS)
        for nb in range(NB):
            for r in range(3):
                for tp in GEO.runs[w][r]:
                    col = nb * NPOOL + tp // POOL
                    PLw[0, w, col] += bh[nb]
                    PLw[1:1 + hi - t0, w, col] += L2[nb, tp, t0:hi]
    for si, tp in enumerate(GEO.specials):
        for nb in range(NB):
            col = nb * NPOOL + tp // POOL
            PLw[0, 0, col] += bh[nb]
            PLw[1:1 + WROWS, 0, col] += L2[nb, tp, 0:WROWS]
            PLw[1:1 + T - tlast, NW - 1, col] += L2[nb, tp, tlast:T]

    # Wpat: [128, NCH, 2, 96] fp32 : delta(s,s') * sconv[3h+nbr, c]/250
    Wp = np.zeros((128, NCH, 2, 96), np.float32)
    for ch, (cs0, P) in enumerate(CHUNKS):
        for p in range(P):
            c, s = divmod(cs0 + p, S)
            for h in range(2):
                for nbr in range(3):
                    Wp[p, ch, h, nbr * S + s] = sconv[3 * h + nbr, c] / 250.0

    gateW = np.zeros((7, MNB), np.float32)
    gateW[:6] = (gate_w.T / NPOOL)
    gateW[6] = gate_b

    expW = np.zeros((7, 3, 96), np.float32)
    for ch3 in range(3):
        for j in range(96):
            eo = ch3 * 96 + j
            e, o = divmod(eo, MNB)
            expW[:6, ch3, j] = exp_w[e, o, :] * escale[e, o]
            expW[6, ch3, j] = exp_b[e, o] * escale[e, o] + ebeta[e, o]

    SEL = np.zeros((96, 3, MNB), np.float32)
    for ch3 in range(3):
        for j in range(96):
            SEL[j, ch3, (ch3 * 96 + j) % MNB] = 1.0

    fcW = np.zeros((49, NPOOL, NCLS), np.float32)
    for p in range(NPOOL):
        for o in range(MNB):
            fcW[o, p, :] = fc_w[:, o * NPOOL + p]
    fcW[48, 0, :] = fc_b

    iden = np.eye(32, dtype=np.float32)

    return {
        "rmat": R.reshape(128, -1).astype(np.float16),
        "rspec": RS.reshape(128, -1).astype(np.float16),
        "plmat": PLw.reshape(128, -1).astype(np.float16),
        "wpat": Wp.reshape(128, -1),
        "gatew": gateW,
        "expw": expW.reshape(7, -1),
        "selm": SEL.reshape(96, -1),
        "fcw": fcW.reshape(49, -1),
        "iden": iden,
        "onesrow": np.ones((1, NPOOL * S), np.float32),
    }


def build_xtw(xcore):
    """Per-core x windows: [128, NW, CS] fp16 (row 0 = ones, rows 1.. = x^T)."""
    xt = np.zeros((128, NW, CS), np.float16)
    xt[0] = 1.0
    xpad = np.zeros((S, C, GEO.tau0[-1] + WROWS), np.float32)
    xpad[:, :, :T] = xcore
    for w in range(NW):
        t0 = GEO.tau0[w]
        blk = xpad[:, :, t0:t0 + WROWS]                   # (S, C, 112)
        xt[1:1 + WROWS, w, :] = blk.transpose(2, 1, 0).reshape(WROWS, CS).astype(np.float16)
    return xt.reshape(128, -1)


# ----------------------------------------------------------------------------
# bass program
# ----------------------------------------------------------------------------

def build_program():
    nc = bacc.Bacc("TRN2", target_bir_lowering=False, debug=False)

    din = {}
    for name, shape, dt in [
        ("xtw", (128, NW * CS), F16),
        ("rmat", (128, NW * NB * 3 * RL), F16),
        ("rspec", (128, 2 * NB * 2), F16),
        ("plmat", (128, NW * MNB), F16),
        ("wpat", (128, NCH * 2 * 96), F32),
        ("gatew", (7, MNB), F32),
        ("expw", (7, 3 * 96), F32),
        ("selm", (96, 3 * MNB), F32),
        ("fcw", (49, NPOOL * NCLS), F32),
        ("iden", (32, 32), F32),
        ("onesrow", (1, NPOOL * S), F32),
    ]:
        din[name] = nc.dram_tensor(name, list(shape), dt, kind="ExternalInput").ap()
    g6d = nc.dram_tensor("g6scratch", [E, S], F32, kind="Internal").ap()
    out1 = nc.dram_tensor("out1", [MNB, NPOOL * S], F32, kind="ExternalOutput").ap()
    out2 = nc.dram_tensor("out2", [S, NCLS], F32, kind="ExternalOutput").ap()

    with tile.TileContext(nc) as tc:
        with (
            tc.tile_pool(name="consts", bufs=1) as cpool,
            tc.tile_pool(name="ps", bufs=4, space="PSUM") as ps_pool,
            tc.tile_pool(name="acc", bufs=1, space="PSUM") as acc_pool,
            tc.tile_pool(name="cc", bufs=1, space="PSUM") as cc_pool,
            tc.tile_pool(name="sb", bufs=2) as sbp,
            tc.tile_pool(name="small", bufs=1) as sm,
        ):
            # ---- load constants ----
            sb = {}
            for name, ap in din.items():
                t = cpool.tile(list(ap.shape), ap.dtype, tag=name)
                nc.sync.dma_start(t[:], ap)
                sb[name] = t
            xtw = sb["xtw"][:].rearrange("p (w n) -> p w n", w=NW)         # [128, NW, 704]
            rmat = sb["rmat"][:].rearrange("p (w n) -> p w n", w=NW)         # [128, NW, 504]
            plm = sb["plmat"][:].rearrange("p (w n) -> p w n", w=NW)
            rsp = sb["rspec"][:].rearrange("p (k n w2) -> p k n w2", k=2, n=NB)
            wpat = sb["wpat"][:].rearrange("p (c h m) -> p c h m", c=NCH, h=2)
            expw = sb["expw"][:].rearrange("p (c m) -> p c m", c=3)
            selm = sb["selm"][:].rearrange("p (c m) -> p c m", c=3)
            fcw = sb["fcw"][:].rearrange("p (k n) -> p k n", k=NPOOL)

            cc_ps = [cc_pool.tile([96, MNB], F32, tag=f"cc{h}", name=f"cc{h}") for h in range(2)]

            # ================= main loop: chunks x windows =================
            for ch, (cs0, P) in enumerate(CHUNKS):
                lin_ps = acc_pool.tile([P, MNB], F32, tag="lin")
                sp_ps = acc_pool.tile([P, NB * 2], F32, tag="sp")
                partials = sbp.tile([128, MNB * SL], F32, tag="partials")
                nc.vector.memset(partials[:], 0.0)
                pview = partials[:].rearrange("p (n sl) -> p n sl", sl=SL)

                for w in range(NW):
                    lhsT = xtw[:, w, cs0:cs0 + P]
                    nc.tensor.matmul(lin_ps[:], lhsT, plm[:, w, :],
                                     start=(w == 0), stop=(w == NW - 1))
                    if w == 0:
                        nc.tensor.matmul(sp_ps[:], lhsT,
                                         rsp[:, 0].rearrange("p n w2 -> p (n w2)"),
                                         start=True, stop=False)
                    if w == NW - 1:
                        nc.tensor.matmul(sp_ps[:], lhsT,
                                         rsp[:, 1].rearrange("p n w2 -> p (n w2)"),
                                         start=False, stop=True)
                    pst = ps_pool.tile([P, NB * TLW], F32, tag="ps", name="pst")
                    nc.tensor.matmul(pst[:], lhsT, rmat[:, w, :], start=True, stop=True)
                    pg = partials[:P].rearrange("p (n x) -> p n x", n=NB)
                    pv = pst[:].rearrange("p (n r j) -> p n r j", n=NB, r=3)
                    s0, D = GEO.call1[w]
                    nc.vector.tensor_reduce(
                        pg[:, :, s0:s0 + 2 * D + 1:D], pv,
                        axis=AX.X, op=ALU.add, apply_absolute_value=True)
                    # tails: pool-crossing corrections (always fp32 PSUM source)
                    for (r, ci, pB, sB, pA, sC) in GEO.tails[w]:
                        tin = pv[:, :, r, ci:RL]
                        nc.vector.tensor_reduce(
                            pg[:, :, pB * SL + sB:pB * SL + sB + 1], tin,
                            axis=AX.X, op=ALU.add, apply_absolute_value=True)
                        nc.vector.tensor_reduce(
                            pg[:, :, pA * SL + sC:pA * SL + sC + 1], tin,
                            axis=AX.X, op=ALU.add, apply_absolute_value=True,
                            negate=True)
                # specials reduce (which=0 -> pool2, which=1 -> pool5)
                for si, pool_id in ((0, 2), (1, 5)):
                    col = pool_id * SL + GEO.sp_slot[pool_id]
                    nc.vector.tensor_reduce(
                        partials[:P].rearrange("p (n x) -> p n x", n=NB)[:, :, col:col + 1],
                        sp_ps[:].rearrange("p (n w2) -> p n w2", w2=2)[:, :, si:si + 1],
                        axis=AX.X, op=ALU.add, apply_absolute_value=True)
                # combine: hpS = sum_slots partials + lin
                hpS = sbp.tile([P, MNB], F32, tag="hpS")
                nc.vector.tensor_reduce(hpS[:], pview[:P], axis=AX.X, op=ALU.add)
                nc.vector.tensor_tensor(hpS[:], hpS[:], lin_ps[:], ALU.add)
                for h in range(2):
                    nc.tensor.matmul(cc_ps[h][:], wpat[:P, ch, h, :], hpS[:],
                                     start=(ch == 0), stop=(ch == NCH - 1))

            # ========================= head =========================
            ccS = [sm.tile([96, MNB], F32, tag=f"ccS{h}", name=f"ccS{h}") for h in range(2)]
            for h in range(2):
                nc.vector.tensor_copy(ccS[h][:], cc_ps[h][:])
            hpT = sm.tile([7, NPOOL * S], F32, tag="hpT")   # rows 0-5 hpoolT, row6 ones
            nc.sync.dma_start(hpT[6:7, :], din["onesrow"])
            for h in range(2):
                for nbr in range(3):
                    nb = 3 * h + nbr
                    src = ccS[h][nbr * S:(nbr + 1) * S, nb * NPOOL:(nb + 1) * NPOOL]  # [32(s), 8(p)]
                    dst = hpT[nb:nb + 1, :].rearrange("one (s p) -> one s p", s=S)
                    nc.sync.dma_start(dst, src)
            # gate tile [7, 32]: rows 0-5 = sum_p hpoolT, row 6 ones
            gt = sm.tile([7, S], F32, tag="gt")
            nc.sync.dma_start(gt[6:7, :], din["onesrow"][:, 0:S])
            nc.vector.tensor_reduce(
                gt[0:6, :], hpT[0:6, :].rearrange("n (s p) -> n s p", s=S),
                axis=AX.X, op=ALU.add)
            glog = ps_pool.tile([S, MNB], F32, tag="ps")
            nc.tensor.matmul(glog[:], gt[:], sb["gatew"][:], start=True, stop=True)
            # softmax
            m1 = sm.tile([S, 1], F32, tag="m1")
            nc.vector.tensor_reduce(m1[:], glog[:], axis=AX.X, op=ALU.max)
            nm1 = sm.tile([S, 1], F32, tag="nm1")
            nc.vector.tensor_scalar_mul(nm1[:], m1[:], -1.0)
            ex = sm.tile([S, MNB], F32, tag="ex")
            nc.scalar.activation(ex[:], glog[:], ACTF.Exp, bias=nm1[:], scale=1.0)
            sm1 = sm.tile([S, 1], F32, tag="sm1")
            nc.vector.tensor_reduce(sm1[:], ex[:], axis=AX.X, op=ALU.add)
            rs1 = sm.tile([S, 1], F32, tag="rs1")
            nc.vector.reciprocal(rs1[:], sm1[:])
            gw = sm.tile([S, MNB], F32, tag="gw")
            nc.vector.tensor_scalar(gw[:], ex[:], rs1[:], None, op0=ALU.mult)
            # top-3 mask
            msum = sm.tile([S, MNB], F32, tag="msum")
            nc.vector.memset(msum[:], 0.0)
            cur = gw
            for it in range(K):
                mx = sm.tile([S, 1], F32, tag=f"mx{it}")
                nc.vector.tensor_reduce(mx[:], cur[:], axis=AX.X, op=ALU.max)
                mk = sm.tile([S, MNB], F32, tag=f"mk{it}")
                nc.vector.tensor_scalar(mk[:], cur[:], mx[:], None, op0=ALU.is_ge)
                nc.vector.tensor_tensor(msum[:], msum[:], mk[:], ALU.add)
                if it < K - 1:
                    nxt = sm.tile([S, MNB], F32, tag=f"cur{it}")
                    big = sm.tile([S, MNB], F32, tag=f"big{it}")
                    nc.vector.tensor_scalar_mul(big[:], mk[:], 1e30)
                    nc.vector.tensor_tensor(nxt[:], cur[:], big[:], ALU.subtract)
                    cur = nxt
            gwm = sm.tile([S, MNB], F32, tag="gwm")
            nc.vector.tensor_tensor(gwm[:], gw[:], msum[:], ALU.mult)
            den = sm.tile([S, 1], F32, tag="den")
            nc.vector.tensor_reduce(den[:], gwm[:], axis=AX.X, op=ALU.add)
            rden = sm.tile([S, 1], F32, tag="rden")
            nc.vector.reciprocal(rden[:], den[:])
            gwn = sm.tile([S, MNB], F32, tag="gwn")
            nc.vector.tensor_scalar(gwn[:], gwm[:], rden[:], None, op0=ALU.mult)
            # gw6T [6, 32] via PE transpose
            g6ps = ps_pool.tile([E, S], F32, tag="ps")
            nc.tensor.matmul(g6ps[:], gwn[:, 0:E], sb["iden"][:], is_transpose=True)
            gw6T = sm.tile([E, S], F32, tag="gw6T")
            nc.vector.tensor_copy(gw6T[:], g6ps[:])
            nc.sync.dma_start(g6d, gw6T[:])
            # experts + mixing
            mix_ps = ps_pool.tile([MNB, NPOOL * S], F32, tag="ps")
            for ch3 in range(3):
                yp_ps = ps_pool.tile([96, NPOOL * S], F32, tag="ps")
                nc.tensor.matmul(yp_ps[:], expw[:, ch3, :], hpT[:], start=True, stop=True)
                ypR = sbp.tile([96, NPOOL * S], F32, tag="ypR")
                nc.scalar.activation(ypR[:], yp_ps[:], ACTF.Relu)
                G = sbp.tile([96, NPOOL * S], F32, tag="G")
                for er in range(2):
                    e = 2 * ch3 + er
                    # G stored (p,s)-major: innermost s is contiguous on both sides
                    srcb = g6d[e:e + 1, :].unsqueeze(1).broadcast_to([48, NPOOL, S])
                    dstb = G[er * 48:(er + 1) * 48, :].rearrange("o (p s) -> o p s", p=NPOOL)
                    nc.sync.dma_start(dstb, srcb)
                Gm = sbp.tile([96, NPOOL * S], F32, tag="Gm")
                ypv = ypR[:].rearrange("o (s p) -> o s p", s=S)
                Gv = G[:].rearrange("o (p s) -> o p s", p=NPOOL).transpose([0, 2, 1])
                Gmv = Gm[:].rearrange("o (s p) -> o s p", s=S)
                nc.vector.tensor_tensor(Gmv, ypv, Gv, ALU.mult)
                nc.tensor.matmul(mix_ps[:], selm[:, ch3, :], Gm[:],
                                 start=(ch3 == 0), stop=(ch3 == 2))
            mixS = sm.tile([49, NPOOL * S], F32, tag="mixS")
            nc.sync.dma_start(mixS[48:49, :], din["onesrow"])
            nc.vector.tensor_copy(mixS[0:48, :], mix_ps[:])
            nc.sync.dma_start(out1, mixS[0:48, :])
            # fc
            lg_ps = ps_pool.tile([S, NCLS], F32, tag="ps")
            mixv = mixS[:].rearrange("k (s p) -> k s p", s=S)
            for p in range(NPOOL):
                nc.tensor.matmul(lg_ps[:], mixv[:, :, p], fcw[:, p, :],
                                 start=(p == 0), stop=(p == NPOOL - 1))
            m2 = sm.tile([S, 1], F32, tag="m2")
            nc.vector.tensor_reduce(m2[:], lg_ps[:], axis=AX.X, op=ALU.max)
            nm2 = sm.tile([S, 1], F32, tag="nm2")
            nc.vector.tensor_scalar_mul(nm2[:], m2[:], -1.0)
            ex2 = sm.tile([S, NCLS], F32, tag="ex2")
            nc.scalar.activation(ex2[:], lg_ps[:], ACTF.Exp, bias=nm2[:], scale=1.0)
            s2 = sm.tile([S, 1], F32, tag="s2")
            nc.vector.tensor_reduce(s2[:], ex2[:], axis=AX.X, op=ALU.add)
            ls2 = sm.tile([S, 1], F32, tag="ls2")
            nc.scalar.activation(ls2[:], s2[:], ACTF.Ln)
            lgp = sm.tile([S, NCLS], F32, tag="lgp")
            nc.vector.tensor_scalar(lgp[:], lg_ps[:], m2[:], ls2[:],
                                    op0=ALU.subtract, op1=ALU.subtract)
            nc.sync.dma_start(out2, lgp[:])

    nc.compile()
    return nc


_NC = None


def _get_nc():
    global _NC
    if _NC is None:
        _NC = build_program()
    return _NC


def kernel(**inputs):
    x = np.asarray(inputs["x"], np.float32)
    consts = build_consts(inputs)
    nc = _get_nc()
    in_maps = []
    for core in range(NCORES):
        m = dict(consts)
        m["xtw"] = build_xtw(x[core * S:(core + 1) * S])
        in_maps.append(m)
    res = run_bass_kernel_spmd(nc, in_maps, list(range(NCORES))).results
    feats = np.zeros((B, MNB * NPOOL), np.float32)
    logp = np.zeros((B, NCLS), np.float32)
    for core in range(NCORES):
        o1 = res[core]["out1"].reshape(MNB, S, NPOOL)      # [o, s, p]
        feats[core * S:(core + 1) * S] = o1.transpose(1, 0, 2).reshape(S, MNB * NPOOL)
        logp[core * S:(core + 1) * S] = res[core]["out2"]
    return feats, logp


def _install_ntff_hook():
    """Recreate antenv.axon_hooks + the ctypes NTFF hook (absent in this image)."""
    import types, ctypes, contextlib
    if "antenv.axon_hooks" in sys.modules:
        return
    so_path = "/opt/axon/libaxon_pjrt.so"
    lib = ctypes.CDLL(so_path)
    if not hasattr(lib, "axon_start_nrt_profile"):
        hook = None
    else:
        lib.axon_start_nrt_profile.argtypes = [ctypes.POINTER(ctypes.c_int64), ctypes.c_size_t]
        lib.axon_start_nrt_profile.restype = ctypes.c_int64
        lib.axon_stop_nrt_profile.argtypes = [ctypes.c_char_p]
        lib.axon_stop_nrt_profile.restype = ctypes.c_int64

        @contextlib.contextmanager
        def hook(output_dir, device_ids):
            import jax
            jax.devices()
            if device_ids:
                ids = (ctypes.c_int64 * len(device_ids))(*device_ids)
                rc = lib.axon_start_nrt_profile(ids, len(device_ids))
            else:
                rc = lib.axon_start_nrt_profile(None, 0)
            if rc != 0:
                raise RuntimeError(f"axon_start_nrt_profile rc={rc}")
            try:
                yield
            finally:
                n = lib.axon_stop_nrt_profile(str(output_dir).encode())
                print(f"profile: {n} file(s) written to {output_dir}", file=sys.stderr)

    mod = types.ModuleType("antenv.axon_hooks")
    mod.get_axon_ntff_profile_hook = lambda: hook
    mod.set_axon_ntff_profile_hook = lambda h: None
    sys.modules["antenv.axon_hooks"] = mod


def profile_run(**inputs):
    """Run with NTFF tracing; returns (exec_time_ns, profile info)."""
    _install_ntff_hook()
    x = np.asarray(inputs["x"], np.float32)
    consts = build_consts(inputs)
    nc = _get_nc()
    in_maps = []
    for core in range(NCORES):
        m = dict(consts)
        m["xtw"] = build_xtw(x[core * S:(core + 1) * S])
        in_maps.append(m)
    import tempfile
    tmpd = tempfile.mkdtemp(prefix="bassprof_")
    r = run_bass_kernel_spmd(nc, in_maps, list(range(NCORES)), trace=True, tmpdir=tmpd)
    return r.exec_time_ns, tmpd
